# revision 132
# baseline (speedup 1.0000x reference)
"""Trainium2 Bass kernel for nn_AdaptiveChebBlock (8 NeuronCores).

Sharding: batch b = core//2 (4 batches), row-half j = core%2 (2048 rows each).
Each core computes its 2048 rows of the dynamic top-k adjacency + Chebyshev
propagation for its batch; pair collectives (AllGather over [2c,2c+1]) exchange
the degree vector and T1.

v3 design (vs. v2 fp16 mask-drain baseline):
- xn is fp8e4m3: all A-build matmuls (row-side candidate strips + AT side)
  run fp8. Candidates/threshold and the shifted adjacency derive from the
  SAME fp8 products, so the top-k mask stays self-consistent.
- shift decomposition instead of a DVE mask drain: the AT-side matmul
  accumulates a rank-1 fp16 term (ones^T x (-thr[row])) into the same PSUM,
  so PSUM holds A^T - thr[row]. The ACT PSUM->SBUF copy applies Relu and
  writes S = relu(A - thr) in fp8 (MT_S). Masked adjacency = S + thr*ind
  with ind = (S > 0), one fp8 tensor_scalar (2x DVE mode) per quad of col
  tiles, mostly on the otherwise-idle GPSIMD/Pool engine.
- Chebyshev combines are fp8 DoubleRow matmuls (j-tile pairs) against MT_S
  and IND with G = dm12*h in fp8: 4x fewer PE cycles than fp16.
- T1 assembly avoids the post-topk broadcast ladder entirely: raw =
  psS + THRb*psI in [h,row] (THRb staged per quarter DURING pass 1), then
  a PE transpose to row-major where cdm1/q1 are per-PARTITION columns read
  straight from dmv. Only T2 uses DRAM-broadcast coefficients (they land
  during the T1 exchange, off the critical path).
- pass 1 drips the AT-build quad-blocks between row strips (PE never
  bursts), windows of 256 rows after every odd tile; relu/ind ops are
  quad-packed [128,4,256] to amortize the fixed ACT/DVE access cost.
- the exchange gap is filled by software-pipelining each combine: my-half
  j-tiles of later strips run while earlier strips wait on the other
  core's half (deg/T1 pair exchanges).
"""
import os, sys
os.environ.setdefault("JAX_PLATFORMS", "")
for _p in ("/root/.axon_site/_ro/trn_rl_repo", "/opt/trn_rl_repo"):
    if os.path.isdir(_p):
        if _p not in sys.path:
            sys.path.insert(0, _p)
        break  # use exactly one copy — mixing versions breaks imports

import numpy as np

import concourse.bass as bass
import concourse.bacc as bacc
import concourse.tile as tile
import concourse.mybir as mybir
import concourse.masks as masks
from concourse.bass_utils import run_bass_kernel_spmd

F32 = mybir.dt.float32
F16 = mybir.dt.float16
F8 = mybir.dt.float8e4
Alu = mybir.AluOpType
Act = mybir.ActivationFunctionType
DRow = mybir.MatmulPerfMode.DoubleRow

KCHEB = 3
TOPK = 32
TELEPORT = 0.1
LN_EPS = 1e-5

# problem shape (hardcoded per spec)
BSZ, NFULL, DDIM = 4, 4096, 128
HDIM, ODIM = 128, 128
N_CORES = 8

NEG_FILL = -1.0e30


class Cfg:
    def __init__(self, n_nodes, n_rows, use_cc, scalars, flags, gelu=True):
        self.n = n_nodes            # nodes this core sees (columns of A)
        self.r = n_rows             # rows this core owns
        self.NT = n_nodes // 128    # node tiles
        self.RT = n_rows // 128     # row tiles
        self.use_cc = use_cc        # emit pair collectives (8-core mode)
        self.c1, self.c2, self.tg = scalars
        # flags: which optional affine params are non-trivial
        self.lng, self.lnb, self.b1, self.b2 = flags
        self.gelu = gelu            # False only for CoreSim (no Gelu in interp)


def _emit(nc, tc, cfg):
    """Emit the whole per-core graph inside TileContext tc."""
    n, r, NT, RT = cfg.n, cfg.r, cfg.NT, cfg.RT
    c1, c2, tg = cfg.c1, cfg.c2, cfg.tg
    gelu_f = Act.Gelu if cfg.gelu else Act.Identity

    # ---- DRAM I/O -------------------------------------------------------
    xf = nc.dram_tensor("xf", [n, DDIM], F32, kind="ExternalInput")       # full batch slice
    xm = nc.dram_tensor("xm", [r, DDIM], F32, kind="ExternalInput")       # my rows
    w1e = nc.dram_tensor("w1e", [DDIM, HDIM], F32, kind="ExternalInput")
    w2e = nc.dram_tensor("w2e", [KCHEB * HDIM, ODIM], F32, kind="ExternalInput")
    lng_e = nc.dram_tensor("lng", [DDIM], F32, kind="ExternalInput")
    lnb_e = nc.dram_tensor("lnb", [DDIM], F32, kind="ExternalInput")
    b1_e = nc.dram_tensor("b1e", [HDIM], F32, kind="ExternalInput")
    b2_e = nc.dram_tensor("b2e", [ODIM], F32, kind="ExternalInput")
    out_e = nc.dram_tensor("out", [r, ODIM], F32, kind="ExternalOutput")

    # DRAM scratch
    thr_dram = nc.dram_tensor("thr_scr", [r], F16)
    qc_dram = nc.dram_tensor("qc_scr", [2 * r], F16)  # q2|cdm2 staging
    dm_in = nc.dram_tensor("dm_in", [r], F32)
    t1_in = nc.dram_tensor("t1_in", [r, HDIM], F16)
    # NOTE: Shared addr_space is rejected for 2-rank groups; plain DRAM works.
    dm_out = nc.dram_tensor("dm_out", [n], F32)
    t1_out = nc.dram_tensor("t1_out", [n, HDIM], F16)
    groups = [[0, 1], [2, 3], [4, 5], [6, 7]]

    import contextlib
    stack = contextlib.ExitStack()
    const = stack.enter_context(tc.tile_pool(name="const", bufs=1))
    persist = stack.enter_context(tc.tile_pool(name="persist", bufs=1))

    id16 = const.tile([128, 128], F16, tag="id16")
    masks.make_identity(nc, id16[:])
    w1s16 = const.tile([DDIM, HDIM], F16, tag="w1s16")
    ones1 = const.tile([1, 128], F16, tag="ones1")
    nc.vector.memset(ones1[:], 1.0)
    if cfg.lng:
        LNG = const.tile([128, DDIM], F32, tag="LNG")
        nc.sync.dma_start(LNG[:], lng_e.ap().partition_broadcast(128))
    if cfg.lnb:
        LNB = const.tile([128, DDIM], F32, tag="LNB")
        nc.sync.dma_start(LNB[:], lnb_e.ap().partition_broadcast(128))
    if cfg.b1:
        B1R = const.tile([128, HDIM], F32, tag="B1R")
        nc.sync.dma_start(B1R[:], b1_e.ap().partition_broadcast(128))

    MT_S = persist.tile([128, NT, r], F8, tag="MT_S")     # relu(A - thr)^T, fp8
    IND = persist.tile([128, NT, r], F8, tag="IND")       # (S > 0), fp8
    w2s = persist.tile([128, KCHEB, ODIM], F16, tag="w2s")
    hROT = persist.tile([128, NT, 128], F16, tag="hROT")  # h fp16, rotated node order
    degM = persist.tile([128, RT], F32, tag="degM")
    dmv = persist.tile([128, 6, RT], F32, tag="dmv")      # [deg|dm12|cdm1|q1|q2|cdm2]
    dm12rot = persist.tile([128, NT], F32, tag="dm12rot")  # dm12, rotated node order
    thrM = persist.tile([128, RT], F16, tag="thrM")       # per-row-tile thresholds
    hTa = persist.tile([128, r], F16, tag="hTa")          # h^T, built at feature end
    THRb = persist.tile([128, r], F16, tag="THRb")  # thr bcast; becomes CT2b in T2
    CB = persist.tile([128, 2, r], F16, tag="CB")   # q2|cdm2 broadcasts (T2)
    iQ2, iCDM2 = range(2)                           # Q2 row becomes QtaT

    # my row-window start (in nodes): core parity picks the half.
    # pid is a RUNTIME scalar: noff/ooff may only appear in DMA source offsets.
    pid = nc.partition_id()
    noff = (pid % 2) * (n - r)        # my half start
    ooff = ((pid + 1) % 2) * (n - r)  # other half start

    # =====================================================================
    # Feature pass: x tiles -> LN -> h=gelu(.@w1) -> h16, xn -> xn^T fp8
    # =====================================================================
    early_stack = contextlib.ExitStack()
    early = early_stack.enter_context(tc.tile_pool(name="early", bufs=1))
    xn8ROT = early.tile([128, NT, 128], F8, tag="xn8ROT")  # my half first
    negthr = early.tile([1, r], F16, tag="negthr")     # -thr by row, partition 0
    candA = early.tile([128, RT, 64], F32, tag="candA")  # top-8 per 512-chunk
    psA = early_stack.enter_context(
        tc.tile_pool(name="p1psA", bufs=3, space="PSUM"))

    # early pass-1 Max strips (rows 0-1023 x cols 0-2047): xn8 for them is
    # ready after the first feature half, so they weave into the second
    # half's emission and fill the otherwise-idle DVE queue there
    astrips = [(t, s) for t in range(8) for s in range(4)]
    aidx = [0]

    def mid_cb(k):
        for _ in range(k):
            if aidx[0] >= len(astrips):
                return
            t, s = astrips[aidx[0]]
            aidx[0] += 1
            ps = psA.tile([128, 512], F32, tag="ps")
            nc.tensor.matmul(ps[:], xn8ROT[:, t, :], xn8ROT[:, 4 * s:4 * s + 4, :])
            nc.vector.max(candA[:, t, s * 8:(s + 1) * 8], ps[:])

    def _moments_arith(pool, nt, bnst, tagp, eps):
        """mean + 1/sqrt(var+eps) (+ sum-of-squares) from bn_stats output."""
        me, mo = bnst[:, :, 1], bnst[:, :, 4]
        m2e, m2o = bnst[:, :, 2], bnst[:, :, 5]
        mu = pool.tile([128, nt], F32, tag=tagp + "_mu", name="mu")
        rstd = pool.tile([128, nt], F32, tag=tagp + "_rstd", name="rstd")
        ssq = pool.tile([128, nt], F32, tag=tagp + "_ssq", name="ssq")
        dl = pool.tile([128, nt], F32, tag=tagp + "_dl", name="dl")
        nc.vector.tensor_tensor(dl[:], me, mo, Alu.subtract)
        nc.vector.tensor_tensor(dl[:], dl[:], dl[:], Alu.mult)       # delta^2
        nc.vector.tensor_tensor(mu[:], me, mo, Alu.add)
        nc.vector.tensor_scalar_mul(mu[:], mu[:], 0.5)               # mean
        nc.vector.tensor_tensor(rstd[:], m2e, m2o, Alu.add)
        nc.vector.scalar_tensor_tensor(rstd[:], dl[:], float(DDIM) / 4.0, rstd[:],
                                       op0=Alu.mult, op1=Alu.add)    # M2 total
        nc.vector.tensor_tensor(ssq[:], mu[:], mu[:], Alu.mult)
        nc.vector.scalar_tensor_tensor(ssq[:], ssq[:], float(DDIM), rstd[:],
                                       op0=Alu.mult, op1=Alu.add)    # sum sq
        nc.vector.tensor_scalar(rstd[:], rstd[:], 1.0 / DDIM, eps,
                                op0=Alu.mult, op1=Alu.add)           # var + eps
        nc.scalar.activation(rstd[:], rstd[:], Act.Sqrt)
        nc.vector.reciprocal(rstd[:], rstd[:])
        return mu, rstd, ssq

    def feature_pass(src, nt, pool, tpool, psum, psum2, mid_cb=None):
        """Node features in ROTATED group order (my half first): the x loads
        use runtime ds() offsets so hROT/xn8ROT are written directly."""
        xall = pool.tile([128, nt, DDIM], F32, tag="ff_xall")
        bnx = pool.tile([128, nt, 6], F32, tag="ff_bnx")
        bnh = pool.tile([128, nt, 6], F32, tag="ff_bnh")
        hg = nt // 2
        for g0 in range(0, nt, 4):
            off, gl = (noff, g0) if g0 < hg else (ooff, g0 - hg)
            nc.sync.dma_start(
                xall[:, g0:g0 + 4, :],
                src.ap()[bass.ds(off, r), :]
                .rearrange("(g p) d -> p g d", p=128)[:, gl:gl + 4, :])
        w1f = pool.tile([DDIM, HDIM], F32, tag="ff_w1f")
        nc.sync.dma_start(w1f[:], w1e[:])
        nc.scalar.copy(w1s16[:], w1f[:])
        nh = nt // 2
        # x-side stats for BOTH halves up front (pure DVE, no gelu deps),
        # with the xln scale/bias folded for the ACT path: xln = rstd*x + nm
        stats = []
        for hf in range(2):
            lo = hf * nh
            for g in range(lo, lo + nh):
                nc.vector.bn_stats(bnx[:, g, :], xall[:, g, :])
            mu, rstd, _ = _moments_arith(pool, nh, bnx[:, lo:lo + nh, :],
                                         f"ffx{hf}", LN_EPS)
            nm = pool.tile([128, nh], F32, tag=f"ff_nm{hf}", name="nm")
            nc.vector.tensor_tensor(nm[:], mu[:], rstd[:], Alu.mult)
            nc.vector.tensor_scalar_mul(nm[:], nm[:], -1.0)
            stats.append((rstd, nm))
        # main per-half chain: xln (ACT) -> h=gelu(.@w1) -> norms -> xn8.
        # mid_cb() drips early pass-1 Max work into the DVE queue, which is
        # otherwise idle here (the chain is ACT/PE-paced).
        for hf in range(2):
            lo = hf * nh
            rstd, nm = stats[hf]
            for g0 in range(lo, lo + nh, 4):
                ph4 = psum2.tile([128, 4, 128], F32, tag="fp_ph4")
                ptx4 = psum.tile([128, 4, 128], F16, tag="fp_ptx4")
                xlnT4 = tpool.tile([128, 4, DDIM], F16, tag="fp_xlnT4")
                for gi in range(4):
                    g = g0 + gi
                    xln = tpool.tile([128, DDIM], F16, tag="fp_xln")
                    if hf == 0:
                        # block 0: DVE is idle here, ACT paces gelu/copies
                        nc.vector.tensor_scalar(xln[:], xall[:, g, :],
                                                rstd[:, g - lo:g - lo + 1],
                                                nm[:, g - lo:g - lo + 1],
                                                op0=Alu.mult, op1=Alu.add)
                    else:
                        # block 1: DVE runs the woven Max strips
                        nc.scalar.activation(xln[:], xall[:, g, :], Act.Identity,
                                             scale=rstd[:, g - lo:g - lo + 1],
                                             bias=nm[:, g - lo:g - lo + 1])
                    if cfg.lng:
                        nc.vector.tensor_tensor(xln[:], xln[:], LNG[:], Alu.mult)
                    if cfg.lnb:
                        nc.vector.tensor_tensor(xln[:], xln[:], LNB[:], Alu.add)
                    nc.tensor.transpose(ptx4[:, gi, :], xln[:], id16[:])
                nc.scalar.copy(xlnT4[:], ptx4[:])
                for gi in range(4):
                    nc.tensor.matmul(ph4[:, gi, :], xlnT4[:, gi, :], w1s16[:])
                    if cfg.b1:
                        nc.vector.tensor_tensor(ph4[:, gi, :], ph4[:, gi, :],
                                                B1R[:], Alu.add)
                nc.scalar.activation(hROT[:, g0:g0 + 4, :], ph4[:], gelu_f)
                for gi in range(4):
                    g = g0 + gi
                    nc.vector.bn_stats(bnh[:, g, :], hROT[:, g, :])

            _, _, ssqh = _moments_arith(pool, nh, bnh[:, lo:lo + nh, :],
                                        f"ffh{hf}", 0.0)
            invh = pool.tile([128, nh], F32, tag=f"ff_invh{hf}", name="invh")
            nc.scalar.activation(invh[:], ssqh[:], Act.Sqrt)
            nc.vector.tensor_scalar_max(invh[:], invh[:], 1e-12)
            nc.vector.reciprocal(invh[:], invh[:])
            for g0 in range(lo, lo + nh, 4):
                xn4 = tpool.tile([128, 4, HDIM], F16, tag="fp_xn4")
                pt4 = psum.tile([128, 4, 128], F16, tag="fp_ptx4")
                for gi in range(4):
                    g = g0 + gi
                    nc.scalar.activation(xn4[:, gi, :], hROT[:, g, :], Act.Copy,
                                         scale=invh[:, g - lo:g - lo + 1])
                    nc.tensor.transpose(pt4[:, gi, :], xn4[:, gi, :], id16[:])
                nc.scalar.copy(xn8ROT[:, g0:g0 + 4, :], pt4[:])
                if hf == 1 and mid_cb is not None:
                    mid_cb(6)

    with tc.tile_pool(name="p0", bufs=1) as p0w, \
         tc.tile_pool(name="p0t", bufs=3) as p0t, \
         tc.tile_pool(name="p0ps", bufs=3, space="PSUM") as p0ps, \
         tc.tile_pool(name="p0ps2", bufs=2, space="PSUM") as p0ps2:
        feature_pass(xf, NT, p0w, p0t, p0ps, p0ps2, mid_cb)
        for k in range(KCHEB):
            w2f = p0w.tile([128, ODIM], F32, tag="w2f")
            nc.sync.dma_start(w2f[:], w2e[k * 128:(k + 1) * 128, :])
            nc.scalar.copy(w2s[:, k, :], w2f[:])
        # h^T strips for the y-stage / Qh / QtaT: built here (ACT is idle
        # during pass-1's top-k, and hTa gates the first T1 assembly)
        for t0 in range(0, RT, 4):
            ptx = p0ps.tile([128, 4, 128], F16, tag="fp_ptx4")
            for ti in range(4):
                nc.tensor.transpose(ptx[:, ti, :], hROT[:, t0 + ti, :], id16[:])
            nc.scalar.copy(hTa[:, t0 * 128:(t0 + 4) * 128], ptx[:])

    # =====================================================================
    # Pass 1: row-side candidates (DVE max8 from PSUM) + shifted AT build
    # =====================================================================
    NQ = n // 512                 # 512-wide strips per row tile
    DEG, DM, CDM1, Q1, Q2, CDM2 = range(6)
    # AT-build row windows: after every odd tile, 256 rows, quad-packed
    # (one [128, 4, 256] relu/ind op per 4 col-tiles). ind8 on Pool for the
    # windows it can drain before the deg collective; the last two windows
    # go to DVE, which is idle once the top-k scan ends.
    WINDOWS = {t: (128 * (t - 1), 256, 'pool' if t < 13 else 'dve')
               for t in range(1, RT, 2)}

    def dm_vectors(sl):
        """deg -> dm12/coefficient columns for row tiles in slice sl. Split
        so tiles 0-14 compute at t14: only one column remains after the last
        top-k (the Sqrt act table stays loaded; Copy/Relu are in every set)."""
        nc.vector.tensor_scalar(dmv[:, DEG, sl], degM[:, sl], c1, c2,
                                op0=Alu.mult, op1=Alu.add)
        nc.scalar.activation(dmv[:, DM, sl], dmv[:, DEG, sl], Act.Sqrt)
        nc.vector.reciprocal(dmv[:, DM, sl], dmv[:, DM, sl])
        nc.vector.tensor_scalar_mul(dmv[:, CDM1, sl], dmv[:, DM, sl], c1)
        nc.vector.tensor_tensor(dmv[:, Q1, sl], dmv[:, DM, sl], dmv[:, DM, sl],
                                Alu.mult)
        nc.vector.tensor_scalar_mul(dmv[:, Q2, sl], dmv[:, Q1, sl], 2.0 * c2)
        nc.vector.tensor_scalar_mul(dmv[:, Q1, sl], dmv[:, Q1, sl], c2)
        nc.vector.tensor_scalar_mul(dmv[:, CDM2, sl], dmv[:, DM, sl], 2.0 * c1)

    def emit_dm_chain(qstage):
        """last dm column + stagings + broadcast ladder + exchange.
        SP-queue order is by consumer priority (HWDGE hops serialize)."""
        dm_vectors(slice(0, RT))
        for i, row in enumerate([Q2, CDM2]):
            nc.vector.tensor_copy(qstage[:, i, :], dmv[:, row, :])
        # DMA ladder, highest-priority consumers first: dm12 halves gate
        # G -> all T1 matmuls; thr tail gates T1 assembly; q2/cdm2 (T2
        # broadcast) land last — T2 assembly happens after the exchange
        nc.sync.dma_start(dm_in.ap().rearrange("(t p) -> p t", p=128), dmv[:, DM, :])
        if cfg.use_cc:
            nc.gpsimd.collective_compute("AllGather", Alu.bypass,
                                         replica_groups=groups,
                                         ins=[dm_in[:].opt()], outs=[dm_out[:].opt()])
        else:
            # stub exchange on the SWDGE path: keeps HWDGE free for the ladder
            nc.gpsimd.dma_start(dm_out[0:r], dm_in[:])
            if n > r:
                nc.gpsimd.dma_start(dm_out[r:n], dm_in[:])
        # my half of dm12rot comes straight from dm_in (no exchange hop)
        nc.sync.dma_start(dm12rot[:, 0:RT],
                          dm_in.ap().rearrange("(g p) -> p g", p=128))
        sl4 = bass.ds(14 * 128, 256)
        nc.sync.dma_start(thr_dram.ap()[sl4].rearrange("(t p) -> p t", p=128),
                          thrM[:, 14:16])
        nc.sync.dma_start(THRb[:, 14 * 128:r],
                          thr_dram.ap()[sl4].partition_broadcast(128))
        nc.sync.dma_start(dm12rot[:, RT:NT],
                          dm_out.ap()[bass.ds(ooff, r)].rearrange("(g p) -> p g", p=128))
        nc.sync.dma_start(qc_dram.ap().rearrange("(i t p) -> p i t", p=128, i=2),
                          qstage[:])
        nc.sync.dma_start(CB[:], qc_dram.ap().partition_broadcast(128))

    dve_ind_early = []   # deferred ind8 slices (DVE): ready before dm12
    dve_ind_late = []    # deferred ind8 slices (DVE): last windows

    with tc.tile_pool(name="p1c", bufs=2) as p1c, \
         tc.tile_pool(name="p1psT", bufs=2, space="PSUM") as psAT, \
         tc.tile_pool(name="p1psq", bufs=1, space="PSUM") as psq:
        pending = []   # deferred AT quad-blocks, dripped between row strips
        while aidx[0] < len(astrips):
            mid_cb(4)  # any early strips the feature weave didn't reach
        for t in range(RT):
            # --- row-side A strips (fp8) + per-512-chunk max8 from PSUM ---
            # (strips s<4 of tiles 0-7 were woven into the feature pass)
            cand = candA[:, t, :]
            top32 = p1c.tile([128, 32], F32, tag="top32")
            for s in range(4 if t < 8 else 0, NQ):
                ps = psA.tile([128, 512], F32, tag="ps")
                nc.tensor.matmul(ps[:], xn8ROT[:, t, :],
                                 xn8ROT[:, 4 * s:4 * s + 4, :])
                nc.vector.max(cand[:, s * 8:(s + 1) * 8], ps[:])
                if pending:
                    pending.pop(0)()   # one AT quad-block between strips
            # --- top-32 of candidates ---
            nc.vector.max(top32[:, 0:8], cand[:])
            for rnd in range(1, 4):
                nc.vector.match_replace(cand[:], top32[:, (rnd - 1) * 8:rnd * 8],
                                        cand[:], NEG_FILL)
                nc.vector.max(top32[:, rnd * 8:(rnd + 1) * 8], cand[:])
            # threshold (clamped at 0: raw-A masking == relu-A masking)
            nc.vector.tensor_scalar_max(thrM[:, t:t + 1], top32[:, 31:32], 0.0)
            # degree = sum(relu(top32)); max(max(v,0),v) == relu(v)
            dsc = p1c.tile([128, 32], F32, tag="dsc")
            nc.vector.scalar_tensor_tensor(dsc[:], top32[:], 0.0, top32[:],
                                           op0=Alu.max, op1=Alu.max,
                                           accum_out=degM[:, t:t + 1])
            # negthr[0, t*128:(t+1)*128] = -thr: PE transpose + ACT scale=-1
            # (no DRAM roundtrip: keeps the AT build ~1us behind the top-k)
            pthr = psq.tile([1, 128], F16, tag="pthr")
            nc.tensor.transpose(pthr[:], thrM[:, t:t + 1], id16[:])
            nc.scalar.activation(negthr[0:1, t * 128:(t + 1) * 128], pthr[:],
                                 Act.Copy, scale=-1.0)
            if t in (3, 7, 11, 13):
                # stage + broadcast thr quarters while HWDGE is idle, so the
                # pass-2 ladder only carries the last quarter
                lo_t, n_t = (12, 2) if t == 13 else (t // 4 * 4, 4)
                slq = bass.ds(lo_t * 128, n_t * 128)
                nc.sync.dma_start(
                    thr_dram.ap()[slq].rearrange("(t p) -> p t", p=128),
                    thrM[:, lo_t:lo_t + n_t])
                nc.sync.dma_start(THRb[:, lo_t * 128:(lo_t + n_t) * 128],
                                  thr_dram.ap()[slq].partition_broadcast(128))
            if t not in WINDOWS:
                continue
            # --- AT build for this row window: A^T - thr, relu'd to fp8 ---
            lo, W, eng = WINDOWS[t]
            gl, gn = lo // 128, W // 128
            if t == RT - 1:
                qstage = p1c.tile([128, 2, RT], F16, tag="qstage")
                emit_dm_chain(qstage)
            # col-tile QUADS: one [128, 4, 256] relu / ind8 op per 4 tiles
            # (amortizes the fixed ACT/DVE access overhead). Deferred: each
            # quad-block is emitted between row strips of later tiles so the
            # PE queue never bursts and starves the DVE Max cadence.
            def make_quad(c0, lo, W, gl, gn, eng, t):
                def emit():
                    pat = psAT.tile([128, 4, 256], F32, tag="pat")
                    for ci in range(4):
                        nc.tensor.matmul(pat[:, ci, :], xn8ROT[:, c0 + ci, :],
                                         xn8ROT[:, gl:gl + gn, :],
                                         start=True, stop=False)
                        nc.tensor.matmul(pat[:, ci, :], ones1[:],
                                         negthr[0:1, lo:lo + W],
                                         start=False, stop=True)
                    if t == 15 and (c0 // 4) % 2 == 1:
                        # last window: DVE idles until the dm12 ladder lands,
                        # so it takes half the relu wall (max(x,0) == relu)
                        nc.vector.tensor_scalar_max(
                            MT_S[:, c0:c0 + 4, lo:lo + W], pat[:], 0.0)
                    else:
                        nc.scalar.activation(MT_S[:, c0:c0 + 4, lo:lo + W],
                                             pat[:], Act.Relu)
                    if eng == 'pool':
                        nc.gpsimd.tensor_scalar(IND[:, c0:c0 + 4, lo:lo + W],
                                                MT_S[:, c0:c0 + 4, lo:lo + W],
                                                0.0, None, op0=Alu.is_gt)
                    elif t == 13:
                        dve_ind_early.append((c0, lo, W))
                    else:
                        dve_ind_late.append((c0, lo, W))
                return emit

            for c0 in range(0, NT, 4):
                pending.append(make_quad(c0, lo, W, gl, gn, eng, t))
            if t == RT - 1:
                while pending:
                    pending.pop(0)()

    early_stack.close()  # xn8ROT/negthr dead after pass 1

    # =====================================================================
    # Pass 2/3: Chebyshev combines against MT_S/IND (fp8 DoubleRow)
    # =====================================================================
    late = stack.enter_context(tc.tile_pool(name="late", bufs=1))
    T1T = late.tile([128, r], F16, tag="T1T")
    P2a = late.tile([128, RT, HDIM], F16, tag="P2a")  # q1*h, row-major
    T2T = late.tile([128, r], F16, tag="T2T")
    T1loc = late.tile([128, RT, HDIM], F16, tag="T1loc")
    G8 = late.tile([128, NT, HDIM], F8, tag="G8")
    G28 = late.tile([128, NT, HDIM], F8, tag="G28")
    xres16 = late.tile([128, RT, DDIM], F16, tag="xres16")

    if cfg.b2:
        B2R = late.tile([128, ODIM], F32, tag="B2R")
        nc.sync.dma_start(B2R[:], b2_e.ap().partition_broadcast(128))

    def combine_pass(G, MT, psC, p2s, asm_fn, pre_last=None):
        """Shifted Chebyshev product: psS = S^T x G, psI = ind^T x G per
        512-row strip (fp8 DoubleRow over j-tile pairs: my-half j first so
        the exchange-dependent other-half lands late), then asm_fn."""
        JH = NT // 4     # j-pairs per node half
        pss = {}

        def half_mms(rg, half):
            if half == 0:
                pss[rg] = (psC.tile([128, 512], F32, tag="psS", name=f"psS{rg}"),
                           psC.tile([128, 512], F32, tag="psI", name=f"psI{rg}"))
            psS, psI = pss[rg]
            rsl = slice(rg * 512, (rg + 1) * 512)
            for jp in range(half * JH, (half + 1) * JH):
                st, sp = jp == 0, jp == NT // 2 - 1
                nc.tensor.matmul(psS[:], G[:, 2 * jp:2 * jp + 2, :],
                                 MT[0][:, 2 * jp:2 * jp + 2, rsl],
                                 start=st, stop=sp, perf_mode=DRow)
                nc.tensor.matmul(psI[:], G[:, 2 * jp:2 * jp + 2, :],
                                 MT[1][:, 2 * jp:2 * jp + 2, rsl],
                                 start=st, stop=sp, perf_mode=DRow)

        def asm(rg):
            psS, psI = pss.pop(rg)
            asm_fn(rg, psS, psI)

        # software-pipelined: my-half j-tiles of later strips run while the
        # other-half / assembly of earlier strips wait on their inputs
        if pre_last is not None:
            # T1: strip 3 (last row window) gated by the DVE-late ind8: keep
            # it last, injecting those ind8 ops (pre_last) just before it
            order = [(0, 'm'), (1, 'm'), (0, 'o'), (0, 'a'),
                     (2, 'm'), (1, 'o'), (1, 'a'), (2, 'o'), (2, 'a'),
                     ('pre',), (3, 'm'), (3, 'o'), (3, 'a')]
        else:
            # T2: other-half gated by the T1 exchange: prefill 3 my-halves
            order = [(0, 'm'), (1, 'm'), (2, 'm'), (0, 'o'), (0, 'a'),
                     (3, 'm'), (1, 'o'), (1, 'a'), (2, 'o'), (2, 'a'),
                     (3, 'o'), (3, 'a')]
        for step in order:
            if step[0] == 'pre':
                if pre_last is not None:
                    pre_last()
            elif step[1] == 'm':
                half_mms(step[0], 0)
            elif step[1] == 'o':
                half_mms(step[0], 1)
            else:
                asm(step[0])

    # ---- pass 2: T1 ----
    t1iv = t1_in.ap().rearrange("(t p) d -> p t d", p=128)
    with tc.tile_pool(name="p2s", bufs=3) as p2s, \
         tc.tile_pool(name="p2ps", bufs=2, space="PSUM") as p2ps, \
         tc.tile_pool(name="p2psT", bufs=2, space="PSUM") as p2psT:
        nc.gpsimd.dma_start(xres16[:], xm.ap().rearrange("(t p) d -> p t d", p=128))

        # my-half G while the deg exchange is in flight, then other half
        for g in range(RT):
            nc.vector.tensor_scalar_mul(G8[:, g, :], hROT[:, g, :],
                                        dm12rot[:, g:g + 1])
        for g in range(RT, NT):
            nc.vector.tensor_scalar_mul(G8[:, g, :], hROT[:, g, :],
                                        dm12rot[:, g:g + 1])

        def dve_inds():
            # last windows' ind8 on DVE: gates only T1 strip 3, so these are
            # injected right before it (everything earlier stays unblocked)
            for c0, lo, W in dve_ind_early + dve_ind_late:
                nc.vector.tensor_scalar(IND[:, c0:c0 + 4, lo:lo + W],
                                        MT_S[:, c0:c0 + 4, lo:lo + W], 0.0,
                                        None, op0=Alu.is_gt)
        # P2a = q1 * h per row tile (per-partition q1 column from dmv)
        for t in range(RT):
            nc.vector.tensor_scalar_mul(P2a[:, t, :], hROT[:, t, :],
                                        dmv[:, Q1, t:t + 1])

        def t1_asm(rg, psS, psI):
            """T1 assembly WITHOUT the broadcast ladder: raw = psS + thr*psI
            in [h,row] (THRb staged during pass 1), transpose to row-major
            where cdm1/q1 are per-PARTITION columns straight from dmv."""
            rsl = slice(rg * 512, (rg + 1) * 512)
            u = p2s.tile([128, 512], F16, tag="u")
            v = p2s.tile([128, 512], F16, tag="v")
            nc.vector.tensor_tensor(v[:], psI[:], THRb[:, rsl], Alu.mult)
            nc.vector.tensor_tensor(u[:], psS[:], v[:], Alu.add)
            pF = p2psT.tile([128, 4, HDIM], F16, tag="ptr4")
            pB = p2psT.tile([128, 4, HDIM], F16, tag="ptb4")
            for ti in range(4):
                nc.tensor.transpose(pF[:, ti, :], u[:, ti * 128:(ti + 1) * 128],
                                    id16[:])
            for ti in range(4):
                t = rg * 4 + ti
                a = p2s.tile([128, HDIM], F16, tag="arow")
                nc.vector.tensor_scalar_mul(a[:], pF[:, ti, :],
                                            dmv[:, CDM1, t:t + 1])
                nc.vector.tensor_tensor(T1loc[:, t, :], a[:], P2a[:, t, :],
                                        Alu.add)
                # transpose back for the y-stage lhsT
                nc.tensor.transpose(pB[:, ti, :], T1loc[:, t, :], id16[:])
            nc.scalar.copy(T1T[:, rsl], pB[:])
            if rg == 3:
                # smaller final piece: the exchange starts sooner
                nc.sync.dma_start(t1iv[:, 12:14, :], T1loc[:, 12:14, :])
                nc.sync.dma_start(t1iv[:, 14:16, :], T1loc[:, 14:16, :])
            else:
                nc.sync.dma_start(t1iv[:, rg * 4:rg * 4 + 4, :],
                                  T1loc[:, rg * 4:rg * 4 + 4, :])

        combine_pass(G8, (MT_S, IND), p2ps, p2s, t1_asm, pre_last=dve_inds)
        # T2 coeffs while the T1 exchange is in flight: THRb -> cdm2*thr,
        # QtaT = q2*T1T - hTa
        nc.vector.tensor_tensor(THRb[:], THRb[:], CB[:, iCDM2, :], Alu.mult)
        nc.vector.tensor_tensor(CB[:, iQ2, :], CB[:, iQ2, :], T1T[:], Alu.mult)
        nc.vector.tensor_tensor(CB[:, iQ2, :], CB[:, iQ2, :], hTa[:], Alu.subtract)

    # T1 exchange
    if cfg.use_cc:
        nc.gpsimd.collective_compute("AllGather", Alu.bypass, replica_groups=groups,
                                     ins=[t1_in[:].opt()], outs=[t1_out[:].opt()])
    else:
        nc.gpsimd.dma_start(t1_out[0:r, :], t1_in[:])
        if n > r:
            nc.gpsimd.dma_start(t1_out[r:n, :], t1_in[:])

    # ---- pass 3: T2 combine + y/output, strip-pipelined ----
    with tc.tile_pool(name="p3G", bufs=1) as p3G, \
         tc.tile_pool(name="p3s", bufs=3) as p3s, \
         tc.tile_pool(name="p3ps", bufs=3, space="PSUM") as p3ps, \
         tc.tile_pool(name="popsY", bufs=2, space="PSUM") as popsY:
        # my half of G2 comes straight from local T1loc (no exchange dep):
        # the first j-tiles of every T2 strip run during the exchange.
        for l in range(RT):
            nc.vector.tensor_scalar_mul(G28[:, l, :], T1loc[:, l, :],
                                        dm12rot[:, l:l + 1])
        T1oth = p3G.tile([128, RT, HDIM], F16, tag="T1oth")
        t1ovr = t1_out.ap()[bass.ds(ooff, r), :].rearrange("(g p) d -> p g d", p=128)
        for g0 in range(0, RT, 4):
            # SWDGE path: lands right behind the exchange on the Pool queue
            nc.gpsimd.dma_start(T1oth[:, g0:g0 + 4, :], t1ovr[:, g0:g0 + 4, :])
            for g in range(g0, g0 + 4):
                nc.vector.tensor_scalar_mul(G28[:, RT + g, :], T1oth[:, g, :],
                                            dm12rot[:, RT + g:RT + g + 1])

        outt_all = p3G.tile([128, RT, ODIM], F32, tag="outt_all")
        oev = out_e.ap().rearrange("(t p) d -> p t d", p=128)

        def t2_asm(rg, psS, psI):
            # broadcast-coefficient assembly (CB lands well before this),
            # then the y quad: 12 matmuls, one 4-wide combine, DMA out
            rsl = slice(rg * 512, (rg + 1) * 512)
            u = p3s.tile([128, 512], F16, tag="u")
            v = p3s.tile([128, 512], F16, tag="v")
            sS = p3s.tile([128, 512], F16, tag="sS")
            sI = p3s.tile([128, 512], F16, tag="sI")
            # ACT (idle here) drains PSUM so the DVE chain runs in 2x mode
            nc.scalar.copy(sS[:], psS[:])
            nc.scalar.copy(sI[:], psI[:])
            nc.vector.tensor_tensor(u[:], sS[:], CB[:, iCDM2, rsl], Alu.mult)
            nc.vector.tensor_tensor(v[:], sI[:], THRb[:, rsl], Alu.mult)
            nc.vector.tensor_tensor(u[:], u[:], v[:], Alu.add)
            nc.vector.tensor_tensor(T2T[:, rsl], u[:], CB[:, iQ2, rsl], Alu.add)
            # last strip split 2+2 so the final out DMA chain starts sooner
            widths = [2, 2] if rg == 3 else [4]
            ti0 = 0
            for w in widths:
                py4 = popsY.tile([128, 4, ODIM], F32, tag="pyY4")
                for ti in range(w):
                    t = rg * 4 + ti0 + ti
                    sl = slice(t * 128, (t + 1) * 128)
                    comps = [hTa[:, sl], T1T[:, sl], T2T[:, sl]]
                    for k in range(KCHEB):
                        nc.tensor.matmul(py4[:, ti, :], comps[k][:], w2s[:, k, :],
                                         start=(k == 0), stop=(k == KCHEB - 1))
                    if cfg.b2:
                        nc.vector.tensor_tensor(py4[:, ti, :], py4[:, ti, :],
                                                B2R[:], Alu.add)
                t0 = rg * 4 + ti0
                nc.vector.scalar_tensor_tensor(outt_all[:, t0:t0 + w, :],
                                               py4[:, 0:w, :], tg,
                                               xres16[:, t0:t0 + w, :],
                                               op0=Alu.mult, op1=Alu.add)
                nc.sync.dma_start(oev[:, t0:t0 + w, :], outt_all[:, t0:t0 + w, :])
                ti0 += w

        combine_pass(G28, (MT_S, IND), p3ps, p3s, t2_asm)

    stack.close()


def build(cfg, num_devices):
    nc = bacc.Bacc("TRN2", target_bir_lowering=False, debug=False,
                   num_devices=num_devices)
    with tile.TileContext(nc) as tc:
        _emit(nc, tc, cfg)
    nc.compile()
    return nc


def _host_scalars(log_tau, gate):
    tau = max(float(np.exp(np.float32(log_tau))), 1e-3)
    c1 = (1.0 - TELEPORT) / tau
    c2 = (1.0 - TELEPORT) / tau + TELEPORT
    tg = float(np.tanh(np.float32(gate)))
    return c1, c2, tg


def _flags(ln_g, ln_b, b1, b2):
    return (not np.all(ln_g == 1.0), not np.all(ln_b == 0.0),
            not np.all(b1 == 0.0), not np.all(b2 == 0.0))


_CACHE = {}


def kernel(x, ln_g, ln_b, w1, b1, w2, b2, log_tau, gate):
    x = np.ascontiguousarray(x, dtype=np.float32)
    assert x.shape == (BSZ, NFULL, DDIM), x.shape
    scalars = _host_scalars(log_tau, gate)
    flags = _flags(np.asarray(ln_g), np.asarray(ln_b), np.asarray(b1), np.asarray(b2))
    key = (scalars, flags)
    if key not in _CACHE:
        cfg = Cfg(NFULL, NFULL // 2, True, scalars, flags)
        _CACHE[key] = (build(cfg, N_CORES), cfg)
    nc, cfg = _CACHE[key]

    r = cfg.r
    base = {
        "w1e": np.ascontiguousarray(w1, np.float32),
        "w2e": np.ascontiguousarray(w2, np.float32),
        "lng": np.ascontiguousarray(ln_g, np.float32),
        "lnb": np.ascontiguousarray(ln_b, np.float32),
        "b1e": np.ascontiguousarray(b1, np.float32),
        "b2e": np.ascontiguousarray(b2, np.float32),
    }
    in_maps = []
    for c in range(N_CORES):
        b, j = c // 2, c % 2
        m = dict(base)
        m["xf"] = x[b]
        m["xm"] = np.ascontiguousarray(x[b, j * r:(j + 1) * r, :])
        in_maps.append(m)

    res = run_bass_kernel_spmd(nc, in_maps, core_ids=list(range(N_CORES)))
    out = np.empty_like(x)
    for c in range(N_CORES):
        b, j = c // 2, c % 2
        out[b, j * r:(j + 1) * r, :] = res.results[c]["out"]
    return out
